# revision 3
# baseline (speedup 1.0000x reference)
"""Trainium2 Bass kernel for nn_ConvBlockFD (frequency-dynamic conv block).

Computation (see header comment in the generating reference):
  y = relu(fdconv2(relu(fdconv1(x))))
where fdconv = per-sample 3x3 conv whose kernel is an attention-weighted
mix of a K=4 kernel bank (bank given by rfft2 coefficients), attention =
softmax(MLP(GAP(input))).

Strategy:
- Data-parallel over batch: B=16 samples, 2 per NeuronCore across 8 cores.
- Host precomputes the irfft2 kernel bank (tiny linear transform) and the
  layer-1 attention + mixed per-sample weights (depends only on x via GAP;
  negligible FLOPs). Layer-2 attention depends on the layer-1 output, so it
  is computed on-device (GAP via activation accum_out, tiny MLP on PE/ACT,
  softmax, then DVE weight mixing from the replicated bank).
- Convs run as 9 shifted matmuls over a zero-ring-padded SBUF image:
  contraction over Cin on partitions, fp16 operands (full PE rate), fp32
  PSUM accumulation, fused ReLU+bias epilogue on the scalar engine.
"""
import os
import numpy as np

import concourse.bacc as bacc
import concourse.mybir as mybir
import concourse.tile as tile
from concourse.bass_utils import run_bass_kernel_spmd

F32 = mybir.dt.float32
F16 = mybir.dt.float16
AF = mybir.ActivationFunctionType
ALU = mybir.AluOpType
AX = mybir.AxisListType

N_CORES = 8
B, Cin, Cout, H, W = 16, 128, 256, 128, 128
S = B // N_CORES          # samples per core
K_NUM, KS = 4, 3
HW = H * W
P = 128                   # partitions / channel group size
G1 = Cin // P             # cin groups layer1 = 1
G2 = Cout // P            # channel groups = 2
ROWS = 4                  # output rows per psum tile (4*128 = 512 = 1 bank)
TPB = 8                   # psum tiles in flight per block
BLK = H // (ROWS * TPB)   # row blocks per (sample, cog) = 4


def build_program():
    nc = bacc.Bacc("TRN2", target_bir_lowering=False, debug=False)

    x_d = nc.dram_tensor("x", [S, Cin, H, W], F32, kind="ExternalInput")
    wd1_d = nc.dram_tensor("wd1", [S, P, 9, Cout], F16, kind="ExternalInput")
    basis2_d = nc.dram_tensor("basis2", [G2, P, K_NUM, 9, Cout], F16, kind="ExternalInput")
    a2w1_d = nc.dram_tensor("a2w1", [G2, P, Cout // 4], F32, kind="ExternalInput")
    a2b1_d = nc.dram_tensor("a2b1", [Cout // 4, 1], F32, kind="ExternalInput")
    a2w2_d = nc.dram_tensor("a2w2", [Cout // 4, K_NUM], F32, kind="ExternalInput")
    a2b2_d = nc.dram_tensor("a2b2", [K_NUM, 1], F32, kind="ExternalInput")
    b1_d = nc.dram_tensor("b1", [G2, P, 1], F32, kind="ExternalInput")
    b2_d = nc.dram_tensor("b2", [G2, P, 1], F32, kind="ExternalInput")
    y_d = nc.dram_tensor("y", [S, G2, P, H, W], F32, kind="ExternalOutput")

    H2 = Cout // 4  # attention hidden = 64

    with tile.TileContext(nc) as tc:
        with (
            tc.tile_pool(name="const", bufs=1) as cpool,
            tc.tile_pool(name="stage", bufs=3) as spool,
            tc.tile_pool(name="outp", bufs=4) as opool,
            tc.tile_pool(name="psum", bufs=8, space="PSUM") as ppool,
        ):
            # ---- persistent SBUF tensors ----
            x_img = cpool.tile([P, H + 2, W + 2], F16, tag="x_img")
            y1 = [cpool.tile([P, H + 2, W + 2], F16, tag=f"y1_{g}", name=f"y1_{g}") for g in range(G2)]
            wd1_t = [cpool.tile([P, 9, Cout], F16, tag=f"wd1_{s}", name=f"wd1_{s}") for s in range(S)]
            basis2_t = [cpool.tile([P, K_NUM, 9, Cout], F16, tag=f"basis2_{g}",
                             name=f"basis2_{g}") for g in range(G2)]
            wd2_t = [cpool.tile([P, 9, Cout], F16, tag=f"wd2_{g}", name=f"wd2_{g}") for g in range(G2)]
            a2w1_t = [cpool.tile([P, H2], F32, tag=f"a2w1_{g}", name=f"a2w1_{g}") for g in range(G2)]
            a2b1_t = cpool.tile([H2, 1], F32, tag="a2b1")
            a2w2_t = cpool.tile([H2, K_NUM], F32, tag="a2w2")
            a2b2_t = cpool.tile([K_NUM, 1], F32, tag="a2b2")
            b1_t = [cpool.tile([P, 1], F32, tag=f"b1_{g}", name=f"b1_{g}") for g in range(G2)]
            b2_t = [cpool.tile([P, 1], F32, tag=f"b2_{g}", name=f"b2_{g}") for g in range(G2)]
            gap_parts = cpool.tile([P, G2 * BLK * TPB], F32, tag="gap_parts")
            gap_t = [cpool.tile([P, 1], F32, tag=f"gap_{g}", name=f"gap_{g}") for g in range(G2)]
            h_t = cpool.tile([H2, 1], F32, tag="h_t")
            logit_t = cpool.tile([K_NUM, 1], F32, tag="logit_t")
            lrow = cpool.tile([1, K_NUM], F32, tag="lrow")
            e_t = cpool.tile([1, K_NUM], F32, tag="e_t")
            sum_t = cpool.tile([1, 1], F32, tag="sum_t")
            rcp_t = cpool.tile([1, 1], F32, tag="rcp_t")
            arow = cpool.tile([1, K_NUM], F32, tag="arow")
            attn_bc = cpool.tile([P, K_NUM], F32, tag="attn_bc")

            # ---- constant loads ----
            for s in range(S):
                nc.sync.dma_start(wd1_t[s][:], wd1_d[s])
            for g in range(G2):
                nc.sync.dma_start(basis2_t[g][:], basis2_d[g])
                nc.sync.dma_start(a2w1_t[g][:], a2w1_d[g])
                nc.sync.dma_start(b1_t[g][:], b1_d[g])
                nc.sync.dma_start(b2_t[g][:], b2_d[g])
            nc.sync.dma_start(a2b1_t[:], a2b1_d[:])
            nc.sync.dma_start(a2w2_t[:], a2w2_d[:])
            nc.sync.dma_start(a2b2_t[:], a2b2_d[:])

            # ---- zero rings (stay zero forever; interiors are overwritten) ----
            for img in [x_img] + y1:
                nc.vector.memset(img[:, 0, :], 0.0)
                nc.vector.memset(img[:, H + 1, :], 0.0)
                nc.vector.memset(img[:, :, 0], 0.0)
                nc.vector.memset(img[:, :, W + 1], 0.0)

            def conv(src_imgs, w_tiles, nsteps, epilogue):
                """3x3 conv: contraction over (cin-group, tap) into psum banks."""
                for cog in range(G2):
                    for blk in range(BLK):
                        ps = [ppool.tile([P, ROWS, W], F32, tag="ps", name=f"ps{i}")
                              for i in range(TPB)]
                        for step in range(nsteps):
                            cig, t = divmod(step, 9)
                            dy, dx = divmod(t, 3)
                            lhsT = w_tiles[cig][:, t, cog * P:(cog + 1) * P]
                            for i in range(TPB):
                                r0 = (blk * TPB + i) * ROWS
                                nc.tensor.matmul(
                                    ps[i][:, :, :],
                                    lhsT,
                                    src_imgs[cig][:, r0 + dy:r0 + dy + ROWS, dx:dx + W],
                                    start=(step == 0),
                                    stop=(step == nsteps - 1),
                                )
                        for i in range(TPB):
                            epilogue(cog, blk * TPB + i, ps[i])

            for s in range(S):
                # ---- load + cast x (fp32 HBM -> fp16 padded SBUF image) ----
                RC = 8  # rows per chunk
                for c in range(H // RC):
                    st = spool.tile([P, RC, W], F32, tag="xstage")
                    nc.sync.dma_start(st[:], x_d[s, :, c * RC:(c + 1) * RC, :])
                    nc.vector.tensor_copy(
                        x_img[:, 1 + c * RC:1 + (c + 1) * RC, 1:1 + W], st[:])

                # ---- conv1 + relu(+bias) + gap accumulation ----
                def epi1(cog, idx, ps):
                    r0 = idx * ROWS
                    col = cog * BLK * TPB + idx
                    nc.scalar.activation(
                        y1[cog][:, r0 + 1:r0 + 1 + ROWS, 1:1 + W], ps[:, :, :],
                        AF.Relu, bias=b1_t[cog][:, 0:1],
                        accum_out=gap_parts[:, col:col + 1])

                conv([x_img], [wd1_t[s]], 9, epi1)

                # ---- layer-2 attention: gap -> MLP -> softmax -> mix wd2 ----
                for g in range(G2):
                    nc.vector.tensor_reduce(
                        gap_t[g][:, 0:1],
                        gap_parts[:, g * BLK * TPB:(g + 1) * BLK * TPB],
                        AX.X, ALU.add)
                h_ps = ppool.tile([H2, 1], F32, tag="ps")
                for g in range(G2):
                    nc.tensor.matmul(h_ps[:, 0:1], a2w1_t[g][:, :], gap_t[g][:, 0:1],
                                     start=(g == 0), stop=(g == G2 - 1))
                nc.scalar.activation(h_t[:, 0:1], h_ps[:, 0:1], AF.Relu,
                                     bias=a2b1_t[:, 0:1])
                l_ps = ppool.tile([K_NUM, 1], F32, tag="ps")
                nc.tensor.matmul(l_ps[:, 0:1], a2w2_t[:, :], h_t[:, 0:1],
                                 start=True, stop=True)
                nc.scalar.activation(logit_t[:, 0:1], l_ps[:, 0:1], AF.Identity,
                                     bias=a2b2_t[:, 0:1])
                # partitions -> free row, exp, normalize, broadcast
                nc.sync.dma_start(lrow[0:1, :], logit_t[:, 0:1])
                nc.scalar.activation(e_t[0:1, :], lrow[0:1, :], AF.Exp,
                                     accum_out=sum_t[0:1, 0:1])
                nc.vector.reciprocal(rcp_t[0:1, 0:1], sum_t[0:1, 0:1])
                nc.vector.tensor_scalar_mul(arow[0:1, :], e_t[0:1, :],
                                            rcp_t[0:1, 0:1])
                nc.gpsimd.partition_broadcast(attn_bc[:, :], arow[0:1, :])
                for g in range(G2):
                    nc.vector.tensor_scalar_mul(
                        wd2_t[g][:, :, :], basis2_t[g][:, 0, :, :],
                        attn_bc[:, 0:1])
                    for k in range(1, K_NUM):
                        nc.vector.scalar_tensor_tensor(
                            wd2_t[g][:, :, :], basis2_t[g][:, k, :, :],
                            attn_bc[:, k:k + 1], wd2_t[g][:, :, :],
                            ALU.mult, ALU.add)

                # ---- conv2 + relu(+bias) -> HBM ----
                def epi2(cog, idx, ps):
                    r0 = idx * ROWS
                    o = opool.tile([P, ROWS, W], F32, tag="o")
                    nc.scalar.activation(o[:, :, :], ps[:, :, :], AF.Relu,
                                         bias=b2_t[cog][:, 0:1])
                    nc.sync.dma_start(y_d[s, cog, :, r0:r0 + ROWS, :], o[:, :, :])

                conv(y1, wd2_t, 2 * 9, epi2)

    nc.compile()
    return nc


_nc_cache = None


def _get_nc():
    global _nc_cache
    if _nc_cache is None:
        _nc_cache = build_program()
    return _nc_cache


def _irfft_basis(w_fr, w_fi):
    return np.fft.irfft2(w_fr + 1j * w_fi, s=(KS, KS), axes=(-2, -1)).astype(np.float32)


def _softmax(v):
    e = np.exp(v - v.max(axis=-1, keepdims=True))
    return e / e.sum(axis=-1, keepdims=True)


def prepare_inputs(inputs):
    """Host precompute + per-core sharding. Returns in_maps list."""
    x = np.ascontiguousarray(np.asarray(inputs['x'], dtype=np.float32))
    w1 = _irfft_basis(np.asarray(inputs['w1_fr']), np.asarray(inputs['w1_fi']))
    w2 = _irfft_basis(np.asarray(inputs['w2_fr']), np.asarray(inputs['w2_fi']))

    # layer-1 attention + per-sample mixed weights (host; depends only on x)
    gap = x.mean((2, 3))
    h = np.maximum(gap @ np.asarray(inputs['a1w1']) + np.asarray(inputs['a1b1']), 0)
    attn1 = _softmax(h @ np.asarray(inputs['a1w2']) + np.asarray(inputs['a1b2']))
    # [K, Co, Ci, ky, kx] -> [K, Ci, t, Co]
    w1T = w1.transpose(0, 2, 3, 4, 1).reshape(K_NUM, Cin, 9, Cout)
    wd1 = np.einsum('bk,kitc->bitc', attn1, w1T).astype(np.float16)  # [B, Ci, 9, Co]

    w2T = w2.transpose(0, 2, 3, 4, 1).reshape(K_NUM, Cout, 9, Cout)  # [K, Ci2, t, Co]
    basis2 = np.ascontiguousarray(
        w2T.transpose(1, 0, 2, 3)).astype(np.float16).reshape(G2, P, K_NUM, 9, Cout)

    a2w1 = (np.asarray(inputs['a2w1'], dtype=np.float32) / HW).reshape(G2, P, Cout // 4)
    a2b1 = np.asarray(inputs['a2b1'], dtype=np.float32).reshape(-1, 1)
    a2w2 = np.ascontiguousarray(np.asarray(inputs['a2w2'], dtype=np.float32))
    a2b2 = np.asarray(inputs['a2b2'], dtype=np.float32).reshape(-1, 1)
    b1 = np.asarray(inputs['b1'], dtype=np.float32).reshape(G2, P, 1)
    b2 = np.asarray(inputs['b2'], dtype=np.float32).reshape(G2, P, 1)

    in_maps = []
    for c in range(N_CORES):
        sl = slice(c * S, (c + 1) * S)
        in_maps.append({
            'x': x[sl],
            'wd1': np.ascontiguousarray(wd1[sl]),
            'basis2': basis2,
            'a2w1': a2w1, 'a2b1': a2b1, 'a2w2': a2w2, 'a2b2': a2b2,
            'b1': b1, 'b2': b2,
        })
    return in_maps


def run(inputs, trace=False, **kwargs):
    nc = _get_nc()
    in_maps = prepare_inputs(inputs)
    res = run_bass_kernel_spmd(nc, in_maps, list(range(N_CORES)),
                               trace=trace, **kwargs)
    y = np.concatenate([r['y'].reshape(S, Cout, H, W) for r in res.results], axis=0)
    return y, res


def kernel(**inputs) -> np.ndarray:
    y, _ = run(inputs, trace=False)
    return y


# revision 4
# speedup vs baseline: 1.0379x; 1.0379x over previous
"""Trainium2 Bass kernel for nn_ConvBlockFD (frequency-dynamic conv block).

Computation:
  y = relu(fdconv2(relu(fdconv1(x))))
where fdconv = per-sample 3x3 conv whose kernel is an attention-weighted
mix of a K=4 kernel bank (bank given by rfft2 coefficients), attention =
softmax(MLP(GAP(input))).

Strategy:
- Data-parallel over batch: B=16 samples, 2 per NeuronCore across 8 cores.
- Host precomputes the irfft2 kernel bank (tiny linear transform) and the
  layer-1 attention + mixed per-sample weights (depends only on x via GAP;
  negligible FLOPs). Layer-2 attention depends on the layer-1 output, so it
  is computed on-device (GAP via activation accum_out, tiny MLP on PE/ACT,
  softmax, then DVE weight mixing from the replicated bank).
- Convs run as 9 shifted matmuls over a zero-ring-padded SBUF image:
  contraction over Cin on partitions, fp16 operands (full PE rate), fp32
  PSUM accumulation, fused ReLU+bias epilogue on the scalar engine.
- x is staged in 8 overlapping row-band tiles so conv1 starts ~4us after
  the first DMA instead of waiting for the whole image; conv1 runs
  tile-major so epilogues pipeline behind the matmul stream; wd2 mixing is
  chunked (3 taps per tile) so conv2 starts as soon as the first chunk of
  mixed weights is ready.
"""
import os
import numpy as np

import concourse.bacc as bacc
import concourse.mybir as mybir
import concourse.tile as tile
from concourse.bass_utils import run_bass_kernel_spmd
from concourse.masks import make_identity

F32 = mybir.dt.float32
F16 = mybir.dt.float16
AF = mybir.ActivationFunctionType
ALU = mybir.AluOpType
AX = mybir.AxisListType

N_CORES = 8
B, Cin, Cout, H, W = 16, 128, 256, 128, 128
S = B // N_CORES          # samples per core
K_NUM, KS = 4, 3
HW = H * W
P = 128                   # partitions / channel group size
G2 = Cout // P            # channel groups = 2
ROWS = 4                  # output rows per psum tile (4*128 = 512 = 1 bank)
TPB = 8                   # psum tiles in flight per conv2 block
BLK = H // (ROWS * TPB)   # conv2 row blocks per (sample, cog) = 4
XB = 8                    # x row-band tiles
XBR = H // XB             # output rows per band = 16
TC = 3                    # taps per wd2 mixing chunk
NCH = 9 // TC             # chunks per cig = 3
H2 = Cout // 4            # attention hidden = 64


def build_program():
    nc = bacc.Bacc("TRN2", target_bir_lowering=False, debug=False)

    x_d = nc.dram_tensor("x", [S, Cin, H, W], F32, kind="ExternalInput")
    wd1_d = nc.dram_tensor("wd1", [S, P, 9, Cout], F16, kind="ExternalInput")
    basis2_d = nc.dram_tensor("basis2", [G2, P, K_NUM, 9, Cout], F16, kind="ExternalInput")
    a2w1_d = nc.dram_tensor("a2w1", [G2, P, H2], F32, kind="ExternalInput")
    a2b1_d = nc.dram_tensor("a2b1", [H2, 1], F32, kind="ExternalInput")
    a2w2_d = nc.dram_tensor("a2w2", [H2, K_NUM], F32, kind="ExternalInput")
    a2b2_d = nc.dram_tensor("a2b2", [K_NUM, 1], F32, kind="ExternalInput")
    b1_d = nc.dram_tensor("b1", [G2, P, 1], F32, kind="ExternalInput")
    b2_d = nc.dram_tensor("b2", [G2, P, 1], F32, kind="ExternalInput")
    y_d = nc.dram_tensor("y", [S, G2, P, H, W], F32, kind="ExternalOutput")

    with tile.TileContext(nc) as tc:
        with (
            tc.tile_pool(name="const", bufs=1) as cpool,
            tc.tile_pool(name="stage", bufs=2) as spool,
            tc.tile_pool(name="outp", bufs=4) as opool,
            tc.tile_pool(name="psum", bufs=8, space="PSUM") as ppool,
        ):
            # ---- persistent SBUF tensors ----
            # x row-band tiles: band b covers padded-image rows
            # [XBR*b, XBR*b + XBR + 1] (local row l = img row XBR*b + l).
            x_band = [cpool.tile([P, XBR + 2, W + 2], F16, tag=f"xb{b}", name=f"xb{b}")
                      for b in range(XB)]
            y1 = [cpool.tile([P, H + 2, W + 2], F16, tag=f"y1_{g}", name=f"y1_{g}")
                  for g in range(G2)]
            wd1_t = [cpool.tile([P, 9, Cout], F16, tag=f"wd1_{s}", name=f"wd1_{s}")
                     for s in range(S)]
            basis2_t = [cpool.tile([P, K_NUM, 9, Cout], F16, tag=f"basis2_{g}",
                                   name=f"basis2_{g}") for g in range(G2)]
            # mixed conv2 weights, chunked by taps for fine-grained deps
            wd2_t = [[cpool.tile([P, TC, Cout], F16, tag=f"wd2_{g}_{c}",
                                 name=f"wd2_{g}_{c}") for c in range(NCH)]
                     for g in range(G2)]
            a2w1_t = [cpool.tile([P, H2], F32, tag=f"a2w1_{g}", name=f"a2w1_{g}")
                      for g in range(G2)]
            a2b1_t = cpool.tile([H2, 1], F32, tag="a2b1")
            a2w2_t = cpool.tile([H2, K_NUM], F32, tag="a2w2")
            a2b2_t = cpool.tile([K_NUM, 1], F32, tag="a2b2")
            b1_t = [cpool.tile([P, 1], F32, tag=f"b1_{g}", name=f"b1_{g}")
                    for g in range(G2)]
            b2_t = [cpool.tile([P, 1], F32, tag=f"b2_{g}", name=f"b2_{g}")
                    for g in range(G2)]
            gap_parts = cpool.tile([P, G2 * XB * (XBR // ROWS)], F32, tag="gap_parts")
            gap_t = [cpool.tile([P, 1], F32, tag=f"gap_{g}", name=f"gap_{g}")
                     for g in range(G2)]
            h_t = cpool.tile([H2, 1], F32, tag="h_t")
            logit_t = cpool.tile([K_NUM, 1], F32, tag="logit_t")
            ident4 = cpool.tile([K_NUM, K_NUM], F32, tag="ident4")
            e_t = cpool.tile([1, K_NUM], F32, tag="e_t")
            sum_t = cpool.tile([1, 1], F32, tag="sum_t")
            rcp_t = cpool.tile([1, 1], F32, tag="rcp_t")
            arow = cpool.tile([1, K_NUM], F32, tag="arow")
            attn_bc = cpool.tile([P, K_NUM], F32, tag="attn_bc")

            # ---- constant loads ----
            for s in range(S):
                nc.sync.dma_start(wd1_t[s][:], wd1_d[s])
            for g in range(G2):
                nc.sync.dma_start(basis2_t[g][:], basis2_d[g])
                nc.sync.dma_start(a2w1_t[g][:], a2w1_d[g])
                nc.sync.dma_start(b1_t[g][:], b1_d[g])
                nc.sync.dma_start(b2_t[g][:], b2_d[g])
            nc.sync.dma_start(a2b1_t[:], a2b1_d[:])
            nc.sync.dma_start(a2w2_t[:], a2w2_d[:])
            nc.sync.dma_start(a2b2_t[:], a2b2_d[:])
            make_identity(nc, ident4[:, :])

            # ---- zero rings (stay zero forever; interiors overwritten) ----
            for b in range(XB):
                nc.vector.memset(x_band[b][:, :, 0], 0.0)
                nc.vector.memset(x_band[b][:, :, W + 1], 0.0)
            nc.vector.memset(x_band[0][:, 0, :], 0.0)
            nc.vector.memset(x_band[XB - 1][:, XBR + 1, :], 0.0)
            for g in range(G2):
                nc.vector.memset(y1[g][:, 0, :], 0.0)
                nc.vector.memset(y1[g][:, H + 1, :], 0.0)
                nc.vector.memset(y1[g][:, :, 0], 0.0)
                nc.vector.memset(y1[g][:, :, W + 1], 0.0)

            def load_band(s, b):
                """DMA + cast x rows for band b (img rows XBR*b .. XBR*b+XBR+1)."""
                g0 = XBR * b
                r_lo = max(g0 - 1, 0)              # x source rows
                r_hi = min(g0 + XBR, H - 1)
                n = r_hi - r_lo + 1
                l_lo = (r_lo + 1) - g0             # local img row of first row
                st = spool.tile([P, XBR + 2, W], F32, tag="xstage", name="xstage")
                nc.sync.dma_start(st[:, :n, :], x_d[s, :, r_lo:r_hi + 1, :])
                nc.vector.tensor_copy(
                    x_band[b][:, l_lo:l_lo + n, 1:1 + W], st[:, :n, :])

            for s in range(S):
                # ---- conv1, band by band, tile-major ----
                for b in range(XB):
                    load_band(s, b)
                    for cog in range(G2):
                        lhsT = [wd1_t[s][:, t, cog * P:(cog + 1) * P] for t in range(9)]
                        for i in range(XBR // ROWS):
                            ps = ppool.tile([P, ROWS, W], F32, tag="ps", name="ps")
                            for t in range(9):
                                dy, dx = divmod(t, 3)
                                l0 = i * ROWS
                                nc.tensor.matmul(
                                    ps[:, :, :], lhsT[t],
                                    x_band[b][:, l0 + dy:l0 + dy + ROWS, dx:dx + W],
                                    start=(t == 0), stop=(t == 8))
                            r0 = b * XBR + i * ROWS
                            col = cog * XB * (XBR // ROWS) + b * (XBR // ROWS) + i
                            nc.scalar.activation(
                                y1[cog][:, r0 + 1:r0 + 1 + ROWS, 1:1 + W],
                                ps[:, :, :], AF.Relu, bias=b1_t[cog][:, 0:1],
                                accum_out=gap_parts[:, col:col + 1])

                # ---- layer-2 attention ----
                npart = XB * (XBR // ROWS)
                for g in range(G2):
                    nc.vector.tensor_reduce(
                        gap_t[g][:, 0:1],
                        gap_parts[:, g * npart:(g + 1) * npart],
                        AX.X, ALU.add)
                h_ps = ppool.tile([H2, 1], F32, tag="ps", name="h_ps")
                for g in range(G2):
                    nc.tensor.matmul(h_ps[:, 0:1], a2w1_t[g][:, :], gap_t[g][:, 0:1],
                                     start=(g == 0), stop=(g == G2 - 1))
                nc.scalar.activation(h_t[:, 0:1], h_ps[:, 0:1], AF.Relu,
                                     bias=a2b1_t[:, 0:1])
                l_ps = ppool.tile([K_NUM, 1], F32, tag="ps", name="l_ps")
                nc.tensor.matmul(l_ps[:, 0:1], a2w2_t[:, :], h_t[:, 0:1],
                                 start=True, stop=True)
                nc.scalar.activation(logit_t[:, 0:1], l_ps[:, 0:1], AF.Identity,
                                     bias=a2b2_t[:, 0:1])
                # transpose [K,1] -> [1,K] on the PE, then softmax
                tr_ps = ppool.tile([1, K_NUM], F32, tag="ps", name="tr_ps")
                nc.tensor.transpose(tr_ps[0:1, :], logit_t[:, 0:1], ident4[:, :])
                nc.scalar.activation(e_t[0:1, :], tr_ps[0:1, :], AF.Exp,
                                     accum_out=sum_t[0:1, 0:1])
                nc.vector.reciprocal(rcp_t[0:1, 0:1], sum_t[0:1, 0:1])
                nc.vector.tensor_scalar_mul(arow[0:1, :], e_t[0:1, :],
                                            rcp_t[0:1, 0:1])
                nc.gpsimd.partition_broadcast(attn_bc[:, :], arow[0:1, :])
                # ---- mix wd2 chunks (3 taps each) ----
                for g in range(G2):
                    for c in range(NCH):
                        nc.vector.tensor_scalar_mul(
                            wd2_t[g][c][:, :, :],
                            basis2_t[g][:, 0, c * TC:(c + 1) * TC, :],
                            attn_bc[:, 0:1])
                        for k in range(1, K_NUM):
                            nc.vector.scalar_tensor_tensor(
                                wd2_t[g][c][:, :, :],
                                basis2_t[g][:, k, c * TC:(c + 1) * TC, :],
                                attn_bc[:, k:k + 1], wd2_t[g][c][:, :, :],
                                ALU.mult, ALU.add)

                # ---- conv2 ----
                def epi2(s, cog, idx, ps):
                    r0 = idx * ROWS
                    o = opool.tile([P, ROWS, W], F32, tag="o", name="o")
                    nc.scalar.activation(o[:, :, :], ps[:, :, :], AF.Relu,
                                         bias=b2_t[cog][:, 0:1])
                    nc.sync.dma_start(y_d[s, cog, :, r0:r0 + ROWS, :], o[:, :, :])

                def lhsT2(cig, t, cog):
                    return wd2_t[cig][t // TC][:, t % TC, cog * P:(cog + 1) * P]

                for cog in range(G2):
                    for blk in range(BLK):
                        last = (cog == G2 - 1 and blk == BLK - 1)
                        if not last:
                            # step-major: overlaps with wd2 chunk mixing
                            ps = [ppool.tile([P, ROWS, W], F32, tag="ps",
                                             name=f"ps{i}") for i in range(TPB)]
                            for step in range(2 * 9):
                                cig, t = divmod(step, 9)
                                dy, dx = divmod(t, 3)
                                for i in range(TPB):
                                    r0 = (blk * TPB + i) * ROWS
                                    nc.tensor.matmul(
                                        ps[i][:, :, :], lhsT2(cig, t, cog),
                                        y1[cig][:, r0 + dy:r0 + dy + ROWS, dx:dx + W],
                                        start=(step == 0), stop=(step == 2 * 9 - 1))
                            for i in range(TPB):
                                epi2(s, cog, blk * TPB + i, ps[i])
                        else:
                            # tile-major: epilogues pipeline behind matmuls
                            for i in range(TPB):
                                ps = ppool.tile([P, ROWS, W], F32, tag="ps", name="ps")
                                for step in range(2 * 9):
                                    cig, t = divmod(step, 9)
                                    dy, dx = divmod(t, 3)
                                    r0 = (blk * TPB + i) * ROWS
                                    nc.tensor.matmul(
                                        ps[:, :, :], lhsT2(cig, t, cog),
                                        y1[cig][:, r0 + dy:r0 + dy + ROWS, dx:dx + W],
                                        start=(step == 0), stop=(step == 2 * 9 - 1))
                                epi2(s, cog, blk * TPB + i, ps)

    nc.compile()
    return nc


_nc_cache = None


def _get_nc():
    global _nc_cache
    if _nc_cache is None:
        _nc_cache = build_program()
    return _nc_cache


def _irfft_basis(w_fr, w_fi):
    return np.fft.irfft2(w_fr + 1j * w_fi, s=(KS, KS), axes=(-2, -1)).astype(np.float32)


def _softmax(v):
    e = np.exp(v - v.max(axis=-1, keepdims=True))
    return e / e.sum(axis=-1, keepdims=True)


def prepare_inputs(inputs):
    """Host precompute + per-core sharding. Returns in_maps list."""
    x = np.ascontiguousarray(np.asarray(inputs['x'], dtype=np.float32))
    w1 = _irfft_basis(np.asarray(inputs['w1_fr']), np.asarray(inputs['w1_fi']))
    w2 = _irfft_basis(np.asarray(inputs['w2_fr']), np.asarray(inputs['w2_fi']))

    # layer-1 attention + per-sample mixed weights (host; depends only on x)
    gap = x.mean((2, 3))
    h = np.maximum(gap @ np.asarray(inputs['a1w1']) + np.asarray(inputs['a1b1']), 0)
    attn1 = _softmax(h @ np.asarray(inputs['a1w2']) + np.asarray(inputs['a1b2']))
    # [K, Co, Ci, ky, kx] -> [K, Ci, t, Co]
    w1T = w1.transpose(0, 2, 3, 4, 1).reshape(K_NUM, Cin, 9, Cout)
    wd1 = np.einsum('bk,kitc->bitc', attn1, w1T).astype(np.float16)  # [B, Ci, 9, Co]

    w2T = w2.transpose(0, 2, 3, 4, 1).reshape(K_NUM, Cout, 9, Cout)  # [K, Ci2, t, Co]
    basis2 = np.ascontiguousarray(
        w2T.transpose(1, 0, 2, 3)).astype(np.float16).reshape(G2, P, K_NUM, 9, Cout)

    a2w1 = (np.asarray(inputs['a2w1'], dtype=np.float32) / HW).reshape(G2, P, H2)
    a2b1 = np.asarray(inputs['a2b1'], dtype=np.float32).reshape(-1, 1)
    a2w2 = np.ascontiguousarray(np.asarray(inputs['a2w2'], dtype=np.float32))
    a2b2 = np.asarray(inputs['a2b2'], dtype=np.float32).reshape(-1, 1)
    b1 = np.asarray(inputs['b1'], dtype=np.float32).reshape(G2, P, 1)
    b2 = np.asarray(inputs['b2'], dtype=np.float32).reshape(G2, P, 1)

    in_maps = []
    for c in range(N_CORES):
        sl = slice(c * S, (c + 1) * S)
        in_maps.append({
            'x': x[sl],
            'wd1': np.ascontiguousarray(wd1[sl]),
            'basis2': basis2,
            'a2w1': a2w1, 'a2b1': a2b1, 'a2w2': a2w2, 'a2b2': a2b2,
            'b1': b1, 'b2': b2,
        })
    return in_maps


def run(inputs, trace=False, **kwargs):
    nc = _get_nc()
    in_maps = prepare_inputs(inputs)
    res = run_bass_kernel_spmd(nc, in_maps, list(range(N_CORES)),
                               trace=trace, **kwargs)
    y = np.concatenate([r['y'].reshape(S, Cout, H, W) for r in res.results], axis=0)
    return y, res


def kernel(**inputs) -> np.ndarray:
    y, _ = run(inputs, trace=False)
    return y
